# revision 31
# baseline (speedup 1.0000x reference)
"""Masked multi-head attention on 8 Trainium2 NeuronCores.

Problem: B=2, H=12, S=2048, D=64 attention with an int32 {0,1} mask
broadcast over heads.  out = softmax(mask ? QK^T/8 : -inf) @ V.

Sharding (8 cores, no cross-core comm):
  core c -> (b = c>>2, head-group hg = (c>>1)&1 -> 6 heads, q-half qh = c&1
  -> 1024 queries).  Each core computes full attention (all 2048 keys) for
  its 6 heads x 1024 queries.

Design (v2) -- the ACT engine's exp throughput is the wall (128 lanes @
1.2 GHz = 153 G elem/s vs 12.6M exps/core), so the kernel splits the
softmax elementwise work across ACT and DVE and strips everything else
off the device:

  - All inputs are packed to fp16 on the host in the exact SBUF-resident
    layouts (Q^T duplicated for both PE row groups, K^T pair-stacked,
    [V | ones] per k-tile); no on-device staging copies or casts.
  - scoresT[k,q] = K^T @ Q row-paired on the PE (two concurrent
    64-contraction matmuls), 16 k-tiles of [128,1024] PSUM per head.
  - exp: k-tiles 0..NA-1 of each head run on ACT (exp(0.125*s) from PSUM,
    fp16 out) followed by a {0,1}-mask multiply on DVE.  k-tiles NA..15
    run entirely on DVE as ONE fused scalar_tensor_tensor:
        probs = bitcast_f16( int16_sat( s * A + Bmask[k,q] ) )
    a Schraudolph exponential: A = 2^10*log2(e)/8 scales the score into
    fp16-exponent units, Bmask = 15360 (= 15<<10, the fp16 exponent bias)
    for kept lanes and -44992 for masked lanes, which drives the
    round-to-nearest-saturating fp32->int16 converter to -32768 = fp16
    -0.0.  The ~+-3% ripple is uniform per softmax row up to a constant
    factor that cancels in the normalization; verified ~1e-3 end-to-end.
  - AV runs transposed: stationary = [V|1] [128k, 65], moving = probs
    [128k, 512q], accumulating out^T [65, 1024] over the 16 k-tiles in
    PSUM.  This needs only 2 LDWEIGHTS per k-tile (vs 16 with probs
    stationary), keeping the PE sequencer queue off the critical path.
    Column 64 accumulates the softmax denominator.
  - The [65,1024] numerator/denominator tile bounces through SBUF (DVE;
    DMA and GpSimd cannot read PSUM) and is DMA'd to HBM raw; the host
    does out = (num/den)^T.  No DVE reciprocal/scale work.
  - Emission: AV lags QK by two k-tile pairs.  The PSUM double-buffer
    already forces exp of tile g-2 to finish before QK tile g issues, so
    a two-pair-lagged AV matmul never blocks the in-order PE queue, the
    PE has work during head 0, and the drain tail is only two pairs.
    All input DMAs are issued up front in deadline order.
"""

import os
import sys

import numpy as np

for _p in ("/opt/trn_rl_repo",):
    if _p not in sys.path and os.path.isdir(_p):
        sys.path.insert(0, _p)

import concourse.bass as bass
import concourse.mybir as mybir
import concourse.tile as tile
from concourse import bacc
from concourse.bass_utils import run_bass_kernel_spmd

FP16 = mybir.dt.float16
F32 = mybir.dt.float32
I16 = mybir.dt.int16

B, H, S, D = 2, 12, 2048, 64
NCORES = 8
HPC = 6        # heads per core
QPC = 1024     # queries per core
KT = S // 128  # 16 k-tiles

NA = 13                    # k-tiles per head on the ACT exp path
NC = KT - NA               # k-tiles per head on the DVE schraudolph path
A_STT = float(1024.0 / np.log(2.0) / 8.0)   # score -> fp16 exponent units
# 15<<10 minus 1024*log2(E[(1+f)2^-f]) ~= 58.8: centers the Schraudolph
# ripple so the approx is unbiased vs the exact-exp tiles (rounded to a
# multiple of 8 so the value is exact in fp16).
B_KEEP = 15304.0
B_MASK = -44992.0          # saturates int16 -> 0x8000 -> fp16 -0.0

_NC_CACHE = None


def build_bass():
    """Build the single-core Bass/Tile program (SPMD across 8 cores)."""
    nc = bacc.Bacc("TRN2", target_bir_lowering=False, debug=False)

    qhd = nc.declare_dram_parameter("qhd", [128, HPC, QPC], FP16, isOutput=False)
    khd = nc.declare_dram_parameter("khd", [128, HPC, QPC], FP16, isOutput=False)
    vtd = nc.declare_dram_parameter("vtd", [128, HPC, KT, 65], FP16, isOutput=False)
    mbd = nc.declare_dram_parameter("mbd", [128, NA, QPC], FP16, isOutput=False)
    bmd = nc.declare_dram_parameter("bmd", [128, NC, QPC], FP16, isOutput=False)
    o = nc.declare_dram_parameter("o", [HPC, 65, QPC], F32, isOutput=True)

    with tile.TileContext(nc) as tc:
        with (
            tc.tile_pool(name="const", bufs=1) as const,
            tc.tile_pool(name="ppa", bufs=8) as ppa,
            tc.tile_pool(name="ppc", bufs=6) as ppc,
            tc.tile_pool(name="outp", bufs=2) as outp,
            tc.tile_pool(name="psc", bufs=2, space="PSUM") as psc,
            tc.tile_pool(name="pav", bufs=2, space="PSUM") as pav,
        ):
            qh = const.tile([128, HPC, QPC], FP16)
            kh = const.tile([128, HPC, QPC], FP16)
            vt = const.tile([128, HPC, KT, 65], FP16)
            mb = const.tile([128, NA, QPC], FP16)
            bm = const.tile([128, NC, QPC], FP16)

            def load_all():
                # Sync issues only what gates the first matmuls (each
                # dma_start costs ~0.65us of issue time on its queue, so
                # fewer, unsplit issues win); the idle GpSimd queue issues
                # the bulk concurrently, in deadline order.
                nc.sync.dma_start(kh[:, 0, :], khd[:, 0, :])
                nc.sync.dma_start(qh[:, 0, :], qhd[:, 0, :])
                nc.sync.dma_start(mb[:, 0:2, :], mbd[:, 0:2, :])
                nc.sync.dma_start(vt[:, 0, :, :], vtd[:, 0, :, :])
                nc.gpsimd.dma_start(mb[:, 2:NA, :], mbd[:, 2:NA, :])
                nc.gpsimd.dma_start(bm[:], bmd[:])
                for h in range(1, HPC):
                    nc.gpsimd.dma_start(qh[:, h, :], qhd[:, h, :])
                    nc.gpsimd.dma_start(kh[:, h, :], khd[:, h, :])
                    nc.gpsimd.dma_start(vt[:, h, :, :], vtd[:, h, :, :])

            def qk_pair(h, j):
                """Score k-tiles 2j, 2j+1 + exp; returns the probs tile(s)."""
                scs = []
                for r in range(2):
                    sc = psc.tile([128, QPC], F32, tag="sc")
                    lo, hi = 64 * r, 64 * r + 64
                    for qc in range(QPC // 512):
                        nc.tensor.matmul(
                            sc[:, qc * 512 : qc * 512 + 512],
                            kh[lo:hi, h, 128 * j : 128 * j + 128],
                            qh[lo:hi, h, qc * 512 : qc * 512 + 512],
                            start=True,
                            stop=True,
                            tile_position=(64 * r, 0),
                        )
                    scs.append(sc)
                if 2 * j + 1 < NA:
                    pr = ppa.tile([128, 2 * QPC], FP16, tag="pa")
                    for r in range(2):
                        nc.scalar.activation(
                            pr[:, r * QPC : (r + 1) * QPC],
                            scs[r][:],
                            mybir.ActivationFunctionType.Exp,
                            scale=0.125,
                        )
                    nc.vector.tensor_mul(
                        pr.rearrange("p (t q) -> p t q", t=2),
                        pr.rearrange("p (t q) -> p t q", t=2),
                        mb[:, 2 * j : 2 * j + 2, :],
                    )
                    return [(pr, 0), (pr, QPC)]
                out = []
                for r in range(2):
                    t = 2 * j + r
                    pc = ppc.tile([128, QPC], FP16, tag="pc")
                    if t < NA:  # odd-NA straggler: exact exp + single mask TT
                        nc.scalar.activation(
                            pc[:],
                            scs[r][:],
                            mybir.ActivationFunctionType.Exp,
                            scale=0.125,
                        )
                        nc.vector.tensor_mul(pc[:], pc[:], mb[:, t, :])
                    else:
                        nc.vector.scalar_tensor_tensor(
                            pc[:].bitcast(I16),
                            scs[r][:],
                            A_STT,
                            bm[:, t - NA, :],
                            mybir.AluOpType.mult,
                            mybir.AluOpType.add,
                        )
                    out.append((pc, 0))
                return out

            def av_tiles(h, t0, probs, avp):
                """AV^T matmuls for k-tiles t0, t0+1 of head h."""
                for i, (pr, off) in enumerate(probs):
                    t = t0 + i
                    for qc in range(QPC // 512):
                        nc.tensor.matmul(
                            avp[0:65, qc * 512 : qc * 512 + 512],
                            vt[:, h, t, :],
                            pr[:, off + qc * 512 : off + qc * 512 + 512],
                            start=(t == 0),
                            stop=(t == KT - 1),
                        )

            def emit_out(h, avp):
                # DMA cannot read PSUM (nor can GpSimd); bounce through SBUF.
                osb = outp.tile([65, QPC], F32, tag="os")
                nc.vector.tensor_copy(osb[:], avp[0:65, :])
                nc.sync.dma_start(o[h], osb[:])

            # Emit: AV lags QK by two k-tile pairs.  The psc double-buffer
            # already forces exp (and the mask TT behind it) to finish tile
            # g-2 before QK tile g can issue, so an AV matmul two pairs back
            # never blocks the in-order PE queue -- and it fills the PE
            # during head 0 and shrinks the drain tail to two pairs.
            avps = {}
            jobs = []  # (h, j, probs-for-pair)

            def emit_av_pair(h, j, probs):
                if j == 0:
                    avp_h = pav.tile([128, QPC], F32, tag="av")
                    avps[h] = avp_h
                av_tiles(h, 2 * j, probs, avps[h])
                if j == KT // 2 - 1:
                    emit_out(h, avps.pop(h))

            load_all()
            for h in range(HPC):
                for j in range(KT // 2):
                    jobs.append((h, j, qk_pair(h, j)))
                    if len(jobs) > 2:
                        emit_av_pair(*jobs.pop(0))
            for job in jobs:
                emit_av_pair(*job)

    nc.compile()
    return nc


def _shard(c, Q, K, V, mask):
    b, hg, qhalf = c >> 2, (c >> 1) & 1, c & 1
    hs = slice(hg * HPC, hg * HPC + HPC)
    qs = slice(qhalf * QPC, qhalf * QPC + QPC)

    # qhd: Q^T per head, duplicated on partitions 0-63 / 64-127.
    qt = Q[b, hs, qs, :].transpose(0, 2, 1).astype(np.float16)  # [6, 64, 1024]
    qhd = np.empty((128, HPC, QPC), dtype=np.float16)
    qhd[0:64] = qt.transpose(1, 0, 2)
    qhd[64:128] = qhd[0:64]

    # khd: K^T pair-stacked: rows 64r+d hold k-tile 2j+r, 128 cols per pair.
    kt = K[b, hs, :, :].transpose(0, 2, 1).astype(np.float16)  # [6, 64, 2048]
    kv = kt.reshape(HPC, 64, KT // 2, 2, 128)
    khd = np.empty((128, HPC, QPC), dtype=np.float16)
    for r in range(2):
        khd[64 * r : 64 * r + 64] = kv[:, :, :, r, :].reshape(HPC, 64, QPC).transpose(1, 0, 2)

    # vtd: [V | ones] per (head, k-tile).
    vtd = np.ones((128, HPC, KT, 65), dtype=np.float16)
    vtd[:, :, :, 0:64] = (
        V[b, hs, :, :].astype(np.float16).reshape(HPC, KT, 128, 64).transpose(2, 0, 1, 3)
    )

    # maskT [2048 k, 1024 q]: {0,1} fp16 for ACT tiles, Bmask for DVE tiles.
    mT = mask[b, 0, qs, :].T  # [2048, 1024] int32
    mk = mT.reshape(KT, 128, QPC)
    mbd = mk[0:NA].transpose(1, 0, 2).astype(np.float16)
    bmd = np.where(mk[NA:KT] != 0, np.float16(B_KEEP), np.float16(B_MASK)).transpose(1, 0, 2)
    return {
        "qhd": np.ascontiguousarray(qhd),
        "khd": np.ascontiguousarray(khd),
        "vtd": np.ascontiguousarray(vtd),
        "mbd": np.ascontiguousarray(mbd),
        "bmd": np.ascontiguousarray(bmd),
    }


def get_nc():
    global _NC_CACHE
    if _NC_CACHE is None:
        _NC_CACHE = build_bass()
    return _NC_CACHE


def kernel(Q, K, V, mask):
    Q = np.asarray(Q, dtype=np.float32)
    K = np.asarray(K, dtype=np.float32)
    V = np.asarray(V, dtype=np.float32)
    mask = np.asarray(mask, dtype=np.int32)

    in_maps = [_shard(c, Q, K, V, mask) for c in range(NCORES)]
    res = run_bass_kernel_spmd(get_nc(), in_maps, list(range(NCORES))).results

    out = np.empty((B, H, S, D), dtype=np.float32)
    for c in range(NCORES):
        b, hg, qhalf = c >> 2, (c >> 1) & 1, c & 1
        oc = res[c]["o"]  # [6, 65, 1024] raw numerator/denominator
        num, den = oc[:, 0:64, :], oc[:, 64:65, :]
        out[b, hg * HPC : hg * HPC + HPC, qhalf * QPC : qhalf * QPC + QPC, :] = (
            num / den
        ).transpose(0, 2, 1)
    return out


# revision 32
# speedup vs baseline: 1.1809x; 1.1809x over previous
"""Masked multi-head attention on 8 Trainium2 NeuronCores.

Problem: B=2, H=12, S=2048, D=64 attention with an int32 {0,1} mask
broadcast over heads.  out = softmax(mask ? QK^T/8 : -inf) @ V.

Sharding (8 cores, no cross-core comm):
  core c -> (b = c>>2, head-group hg = (c>>1)&1 -> 6 heads, q-half qh = c&1
  -> 1024 queries).  Each core computes full attention (all 2048 keys) for
  its 6 heads x 1024 queries.

Design (v2) -- the ACT engine's exp throughput is the wall (128 lanes @
1.2 GHz = 153 G elem/s vs 12.6M exps/core), so the kernel splits the
softmax elementwise work across ACT and DVE and strips everything else
off the device:

  - All inputs are packed to fp16 on the host in the exact SBUF-resident
    layouts (Q^T duplicated for both PE row groups, K^T pair-stacked,
    [V | ones] per k-tile); no on-device staging copies or casts.
  - scoresT[k,q] = K^T @ Q row-paired on the PE (two concurrent
    64-contraction matmuls), 16 k-tiles of [128,1024] PSUM per head.
  - exp: k-tiles 0..NA-1 of each head run on ACT (exp(0.125*s) from PSUM,
    fp16 out) followed by a {0,1}-mask multiply on DVE.  k-tiles NA..15
    run entirely on DVE as ONE fused scalar_tensor_tensor:
        probs = bitcast_f16( int16_sat( s * A + Bmask[k,q] ) )
    a Schraudolph exponential: A = 2^10*log2(e)/8 scales the score into
    fp16-exponent units, Bmask = 15360 (= 15<<10, the fp16 exponent bias)
    for kept lanes and -44992 for masked lanes, which drives the
    round-to-nearest-saturating fp32->int16 converter to -32768 = fp16
    -0.0.  The ~+-3% ripple is uniform per softmax row up to a constant
    factor that cancels in the normalization; verified ~1e-3 end-to-end.
  - AV runs transposed: stationary = [V|1] [128k, 65], moving = probs
    [128k, 512q], accumulating out^T [65, 1024] over the 16 k-tiles in
    PSUM.  This needs only 2 LDWEIGHTS per k-tile (vs 16 with probs
    stationary), keeping the PE sequencer queue off the critical path.
    Column 64 accumulates the softmax denominator.
  - The [65,1024] numerator/denominator tile bounces through SBUF (DVE;
    DMA and GpSimd cannot read PSUM) and is DMA'd to HBM raw; the host
    does out = (num/den)^T.  No DVE reciprocal/scale work.
  - Emission: AV lags QK by two k-tile pairs.  The PSUM double-buffer
    already forces exp of tile g-2 to finish before QK tile g issues, so
    a two-pair-lagged AV matmul never blocks the in-order PE queue, the
    PE has work during head 0, and the drain tail is only two pairs.
    All input DMAs are issued up front in deadline order.
"""

import os
import sys

import numpy as np

for _p in ("/opt/trn_rl_repo",):
    if _p not in sys.path and os.path.isdir(_p):
        sys.path.insert(0, _p)

import concourse.bass as bass
import concourse.mybir as mybir
import concourse.tile as tile
from concourse import bacc
from concourse.bass_utils import run_bass_kernel_spmd

FP16 = mybir.dt.float16
F32 = mybir.dt.float32
I16 = mybir.dt.int16

B, H, S, D = 2, 12, 2048, 64
NCORES = 8
HPC = 6        # heads per core
QPC = 1024     # queries per core
KT = S // 128  # 16 k-tiles

NA = 13                    # k-tiles per head on the ACT exp path
NC = KT - NA               # k-tiles per head on the DVE schraudolph path
A_STT = float(1024.0 / np.log(2.0) / 8.0)   # score -> fp16 exponent units
# 15<<10 minus 1024*log2(E[(1+f)2^-f]) ~= 58.8: centers the Schraudolph
# ripple so the approx is unbiased vs the exact-exp tiles (rounded to a
# multiple of 8 so the value is exact in fp16).
B_KEEP = 15304.0
B_MASK = -44992.0          # saturates int16 -> 0x8000 -> fp16 -0.0

_NC_CACHE = None


def build_bass():
    """Build the single-core Bass/Tile program (SPMD across 8 cores)."""
    nc = bacc.Bacc("TRN2", target_bir_lowering=False, debug=False)

    qhd = nc.declare_dram_parameter("qhd", [128, HPC, QPC], FP16, isOutput=False)
    khd = nc.declare_dram_parameter("khd", [128, HPC, QPC], FP16, isOutput=False)
    vtd = nc.declare_dram_parameter("vtd", [128, HPC, KT, 65], FP16, isOutput=False)
    mbd = nc.declare_dram_parameter("mbd", [128, NA, QPC], FP16, isOutput=False)
    bmd = nc.declare_dram_parameter("bmd", [128, NC, QPC], FP16, isOutput=False)
    o = nc.declare_dram_parameter("o", [HPC, 65, QPC], F32, isOutput=True)

    with tile.TileContext(nc) as tc:
        with (
            tc.tile_pool(name="const", bufs=1) as const,
            tc.tile_pool(name="ppa", bufs=8) as ppa,
            tc.tile_pool(name="ppc", bufs=6) as ppc,
            tc.tile_pool(name="outp", bufs=2) as outp,
            tc.tile_pool(name="psc", bufs=2, space="PSUM") as psc,
            tc.tile_pool(name="pav", bufs=2, space="PSUM") as pav,
        ):
            qh = const.tile([128, HPC, QPC], FP16)
            kh = const.tile([128, HPC, QPC], FP16)
            vt = const.tile([128, HPC, KT, 65], FP16)
            mb = const.tile([128, NA, QPC], FP16)
            bm = const.tile([128, NC, QPC], FP16)

            def load_all():
                # Sync issues only what gates the first matmuls (each
                # dma_start costs ~0.65us of issue time on its queue, so
                # fewer, unsplit issues win); the idle GpSimd queue issues
                # the bulk concurrently, in deadline order.
                nc.sync.dma_start(kh[:, 0, :], khd[:, 0, :])
                nc.sync.dma_start(qh[:, 0, :], qhd[:, 0, :])
                nc.sync.dma_start(mb[:, 0:2, :], mbd[:, 0:2, :])
                nc.sync.dma_start(vt[:, 0, :, :], vtd[:, 0, :, :])
                nc.sync.dma_start(mb[:, 2:NA, :], mbd[:, 2:NA, :])
                nc.sync.dma_start(bm[:], bmd[:])
                for h in range(1, HPC):
                    nc.sync.dma_start(qh[:, h, :], qhd[:, h, :])
                    nc.sync.dma_start(kh[:, h, :], khd[:, h, :])
                    nc.sync.dma_start(vt[:, h, :, :], vtd[:, h, :, :])

            def qk_pair(h, j):
                """Score k-tiles 2j, 2j+1 + exp; returns the probs tile(s)."""
                scs = []
                for r in range(2):
                    sc = psc.tile([128, QPC], F32, tag="sc")
                    lo, hi = 64 * r, 64 * r + 64
                    for qc in range(QPC // 512):
                        nc.tensor.matmul(
                            sc[:, qc * 512 : qc * 512 + 512],
                            kh[lo:hi, h, 128 * j : 128 * j + 128],
                            qh[lo:hi, h, qc * 512 : qc * 512 + 512],
                            start=True,
                            stop=True,
                            tile_position=(64 * r, 0),
                        )
                    scs.append(sc)
                if 2 * j + 1 < NA:
                    pr = ppa.tile([128, 2 * QPC], FP16, tag="pa")
                    for r in range(2):
                        nc.scalar.activation(
                            pr[:, r * QPC : (r + 1) * QPC],
                            scs[r][:],
                            mybir.ActivationFunctionType.Exp,
                            scale=0.125,
                        )
                    nc.vector.tensor_mul(
                        pr.rearrange("p (t q) -> p t q", t=2),
                        pr.rearrange("p (t q) -> p t q", t=2),
                        mb[:, 2 * j : 2 * j + 2, :],
                    )
                    return [(pr, 0), (pr, QPC)]
                out = []
                for r in range(2):
                    t = 2 * j + r
                    pc = ppc.tile([128, QPC], FP16, tag="pc")
                    if t < NA:  # odd-NA straggler: exact exp + single mask TT
                        nc.scalar.activation(
                            pc[:],
                            scs[r][:],
                            mybir.ActivationFunctionType.Exp,
                            scale=0.125,
                        )
                        nc.vector.tensor_mul(pc[:], pc[:], mb[:, t, :])
                    else:
                        nc.vector.scalar_tensor_tensor(
                            pc[:].bitcast(I16),
                            scs[r][:],
                            A_STT,
                            bm[:, t - NA, :],
                            mybir.AluOpType.mult,
                            mybir.AluOpType.add,
                        )
                    out.append((pc, 0))
                return out

            def av_tiles(h, t0, probs, avp):
                """AV^T matmuls for k-tiles t0, t0+1 of head h."""
                for i, (pr, off) in enumerate(probs):
                    t = t0 + i
                    for qc in range(QPC // 512):
                        nc.tensor.matmul(
                            avp[0:65, qc * 512 : qc * 512 + 512],
                            vt[:, h, t, :],
                            pr[:, off + qc * 512 : off + qc * 512 + 512],
                            start=(t == 0),
                            stop=(t == KT - 1),
                        )

            def emit_out(h, avp):
                # DMA cannot read PSUM (nor can GpSimd); bounce through SBUF.
                osb = outp.tile([65, QPC], F32, tag="os")
                nc.vector.tensor_copy(osb[:], avp[0:65, :])
                nc.sync.dma_start(o[h], osb[:])

            # Emit: AV lags QK by two k-tile pairs.  The psc double-buffer
            # already forces exp (and the mask TT behind it) to finish tile
            # g-2 before QK tile g can issue, so an AV matmul two pairs back
            # never blocks the in-order PE queue -- and it fills the PE
            # during head 0 and shrinks the drain tail to two pairs.
            avps = {}
            jobs = []  # (h, j, probs-for-pair)

            def emit_av_pair(h, j, probs):
                if j == 0:
                    avp_h = pav.tile([128, QPC], F32, tag="av")
                    avps[h] = avp_h
                av_tiles(h, 2 * j, probs, avps[h])
                if j == KT // 2 - 1:
                    emit_out(h, avps.pop(h))

            load_all()
            for h in range(HPC):
                for j in range(KT // 2):
                    jobs.append((h, j, qk_pair(h, j)))
                    if len(jobs) > 2:
                        emit_av_pair(*jobs.pop(0))
            for job in jobs:
                emit_av_pair(*job)

    nc.compile()
    return nc


def _shard(c, Q, K, V, mask):
    b, hg, qhalf = c >> 2, (c >> 1) & 1, c & 1
    hs = slice(hg * HPC, hg * HPC + HPC)
    qs = slice(qhalf * QPC, qhalf * QPC + QPC)

    # qhd: Q^T per head, duplicated on partitions 0-63 / 64-127.
    qt = Q[b, hs, qs, :].transpose(0, 2, 1).astype(np.float16)  # [6, 64, 1024]
    qhd = np.empty((128, HPC, QPC), dtype=np.float16)
    qhd[0:64] = qt.transpose(1, 0, 2)
    qhd[64:128] = qhd[0:64]

    # khd: K^T pair-stacked: rows 64r+d hold k-tile 2j+r, 128 cols per pair.
    kt = K[b, hs, :, :].transpose(0, 2, 1).astype(np.float16)  # [6, 64, 2048]
    kv = kt.reshape(HPC, 64, KT // 2, 2, 128)
    khd = np.empty((128, HPC, QPC), dtype=np.float16)
    for r in range(2):
        khd[64 * r : 64 * r + 64] = kv[:, :, :, r, :].reshape(HPC, 64, QPC).transpose(1, 0, 2)

    # vtd: [V | ones] per (head, k-tile).
    vtd = np.ones((128, HPC, KT, 65), dtype=np.float16)
    vtd[:, :, :, 0:64] = (
        V[b, hs, :, :].astype(np.float16).reshape(HPC, KT, 128, 64).transpose(2, 0, 1, 3)
    )

    # maskT [2048 k, 1024 q]: {0,1} fp16 for ACT tiles, Bmask for DVE tiles.
    mT = mask[b, 0, qs, :].T  # [2048, 1024] int32
    mk = mT.reshape(KT, 128, QPC)
    mbd = mk[0:NA].transpose(1, 0, 2).astype(np.float16)
    bmd = np.where(mk[NA:KT] != 0, np.float16(B_KEEP), np.float16(B_MASK)).transpose(1, 0, 2)
    return {
        "qhd": np.ascontiguousarray(qhd),
        "khd": np.ascontiguousarray(khd),
        "vtd": np.ascontiguousarray(vtd),
        "mbd": np.ascontiguousarray(mbd),
        "bmd": np.ascontiguousarray(bmd),
    }


def get_nc():
    global _NC_CACHE
    if _NC_CACHE is None:
        _NC_CACHE = build_bass()
    return _NC_CACHE


def kernel(Q, K, V, mask):
    Q = np.asarray(Q, dtype=np.float32)
    K = np.asarray(K, dtype=np.float32)
    V = np.asarray(V, dtype=np.float32)
    mask = np.asarray(mask, dtype=np.int32)

    in_maps = [_shard(c, Q, K, V, mask) for c in range(NCORES)]
    res = run_bass_kernel_spmd(get_nc(), in_maps, list(range(NCORES))).results

    out = np.empty((B, H, S, D), dtype=np.float32)
    for c in range(NCORES):
        b, hg, qhalf = c >> 2, (c >> 1) & 1, c & 1
        oc = res[c]["o"]  # [6, 65, 1024] raw numerator/denominator
        num, den = oc[:, 0:64, :], oc[:, 64:65, :]
        out[b, hg * HPC : hg * HPC + HPC, qhalf * QPC : qhalf * QPC + QPC, :] = (
            num / den
        ).transpose(0, 2, 1)
    return out
